# revision 12
# baseline (speedup 1.0000x reference)
"""Trainium2 Bass kernel for CrossAttentionFusion (fp8 DoubleRow version).

Reference computation (shapes hardcoded):
  B=4, C=256, H=W=128, N=16384, CHUNK=2048, nchunks=8.
  q  = image_features  reshaped to (B, nchunks, CHUNK, C)
  kv = lidar_features  reshaped to (B, nchunks, CHUNK, C)
  per (b, chunk): out = softmax(q @ kv.T / sqrt(C)) @ kv
  output = w0 * image_features + w1 * fused,  w = softmax(modality_weights)

Sharding: the 32 independent (b, chunk) pairs are split 4-per-core across
8 NeuronCores (data parallel over batch x chunk; no communication).

fp8 strategy: Q/KV quantized to fp8e4 (e4m3, max 240) on host; both
matmuls run in MatmulPerfMode.DoubleRow (K=256 per instruction, 2 fp8
MACs/cell/cycle).  P = exp(s/sqrt(C) - 3) is produced in fp8: the -3
bias keeps max P ~17 < 240 and cancels in the softmax ratio because the
row-sum is scaled identically.

Per-core structure, per (b, chunk) pair:
  1. mm1 j-outer: for each k-tile j, ONE DoubleRow weight load (KV_j),
     then 4 panel matmuls -> S^T in PSUM (2-bank [128,2,512] tiles).
  2. exp split across two engines: most tiles on ACT (exp -> fp8,
     1024-wide batched), the rest on DVE via a Schraudolph integer exp
     (custom DVE op relu(x*C0+C1) -> int8, bitcast to fp8e4).
  3. mm2 DoubleRow: (q=128p, 257f) = P^T.T @ [KV | 1/w1] over 8 j-pairs.
     The ones column is pre-scaled by 1/w1 so recip gives w1/rowsum.
  4. Epilogue: DVE recip + normalize (g, bf16); GpSimd fuse
     out = qt*w0 + g (qt bf16); DMA out per panel.
"""

import numpy as np

B, C, H, W = 4, 256, 128, 128
N = H * W
CHUNK = 2048
NCHUNKS = N // CHUNK         # 8
NCORES = 8
PAIRS = B * NCHUNKS          # 32
PPC = PAIRS // NCORES        # 4 pairs (chunks) per core
CT = C // 128                # 2 c-tiles
KT = CHUNK // 128            # 16 k-tiles
QT = CHUNK // 128            # 16 q-tiles
PAN = 512                    # q panel width
NPAN = CHUNK // PAN          # 4 panels
KCS = 272                    # kc tile stride (257 cols used)
JP = KT // 2                 # 8 j-pairs for DoubleRow mm2
SCALE = 1.0 / float(np.sqrt(C))
EBIAS = -3.0                 # exp bias: keeps max P < fp8e4 max (240)
LOG2E_8 = 8.0 / float(np.log(2.0))          # 2^3 / ln2 for e4m3 int exp
EXPI8_S0 = SCALE * LOG2E_8                  # psS scale
EXPI8_SHIFT = -0.4                          # Schraudolph centering
EXPI8_S1 = 56.0 + EBIAS * LOG2E_8 + EXPI8_SHIFT

_BUILD_CACHE = {}
_EXPI8_OP = None


def _register_expi8():
    """Register a custom DVE op: out = relu(in*C0 + C1), written as int8.

    With C0/C1 set per the Schraudolph trick, the int8 bit pattern IS
    exp(scale*s + bias) in fp8e4 (bitcast), so the DVE can compute exp
    tiles in a single pass and share the softmax work with ACT.
    """
    global _EXPI8_OP
    if _EXPI8_OP is not None:
        return _EXPI8_OP
    import concourse.dve_ops as dve_ops
    from concourse.bass import dve_ver_for
    from concourse.dve_spec import C0, C1, Spec, Src0, lower, relu
    from concourse.dve_uop import DveOpSpec

    name = "EXP_I8_SCHRAUDOLPH"
    for op in dve_ops.OPS:
        if op.name == name:
            _EXPI8_OP = op
            return op

    spec = Spec(
        body=relu(Src0 * C0 + C1),
        reference=lambda in0, in1, c0, c1, c2: np.maximum(
            in0.astype(np.float32) * c0 + c1, 0.0
        ),
    )
    opcode = dve_ops._CUSTOM_DVE_ROW_BASE + len(dve_ops.OPS)
    shas = {}
    for ver in ("v3", "v4"):
        try:
            uops = lower(spec, ver=ver)
            shas[ver] = DveOpSpec(
                name=name, opcode=opcode, uops=uops, rd1_en=False
            ).sha(ver)
        except Exception:
            pass
    op = dve_ops.DveOp(name, spec, subdim=False, uops_sha=shas)
    dve_ops.OPS.append(op)
    dve_ops._SUB_OPCODE_FOR_NAME[name] = opcode
    _EXPI8_OP = op
    return op


def _use_dve(j: int, pp: int) -> bool:
    # ~1/3 of the 32 (j, pan-pair) exp tiles per pair go to the DVE
    return (2 * j + pp) % 3 == 2


def _build(w0: float, w1: float):
    from contextlib import ExitStack

    import concourse.bass as bass
    import concourse.tile as tile
    from concourse import bacc, mybir

    expi8 = _register_expi8()

    f32 = mybir.dt.float32
    bf16 = mybir.dt.bfloat16
    f8 = mybir.dt.float8e4
    i8 = mybir.dt.int8
    DR = mybir.MatmulPerfMode.DoubleRow
    Exp = mybir.ActivationFunctionType.Exp
    mult = mybir.AluOpType.mult
    add = mybir.AluOpType.add

    nc = bacc.Bacc("TRN2", target_bir_lowering=False, debug=False)
    qt_d = nc.dram_tensor("qt_sh", (PPC, CHUNK, C), bf16, kind="ExternalInput")
    qb_d = nc.dram_tensor("qb_sh", (PPC, C, CHUNK), f8, kind="ExternalInput")
    kvb_d = nc.dram_tensor("kvb_sh", (PPC, C, CHUNK), f8, kind="ExternalInput")
    kc_d = nc.dram_tensor("kc_sh", (PPC, 128, KT * KCS), f8, kind="ExternalInput")
    out_d = nc.dram_tensor("out_sh", (PPC, CHUNK, C), bf16, kind="ExternalOutput")

    with ExitStack() as ctx:
        tc = ctx.enter_context(tile.TileContext(nc))
        po_qt = ctx.enter_context(tc.tile_pool(name="qt", bufs=2))
        po_qb = ctx.enter_context(tc.tile_pool(name="qb", bufs=2))
        po_kvb = ctx.enter_context(tc.tile_pool(name="kvb", bufs=2))
        po_kc = ctx.enter_context(tc.tile_pool(name="kc", bufs=2))
        po_pt = ctx.enter_context(tc.tile_pool(name="pt", bufs=2))
        po_out = ctx.enter_context(tc.tile_pool(name="outs", bufs=2))
        po_g = ctx.enter_context(tc.tile_pool(name="g", bufs=4))
        po_r = ctx.enter_context(tc.tile_pool(name="r", bufs=4))
        po_psS = ctx.enter_context(tc.tile_pool(name="psS", bufs=3, space="PSUM"))
        po_psO = ctx.enter_context(tc.tile_pool(name="psO", bufs=2, space="PSUM"))
        po_const = ctx.enter_context(tc.tile_pool(name="consts", bufs=1))

        ebias = po_const.tile([128, 1], f32, name="ebias")
        nc.gpsimd.memset(ebias[:], EBIAS)

        chunk_tiles = {}

        def emit_loads(p):
            qb = po_qb.tile([128, CT, CHUNK], f8, name="qb")
            kvb = po_kvb.tile([128, CT, CHUNK], f8, name="kvb")
            qt = po_qt.tile([128, QT * C], bf16, name="qt")
            half = CHUNK // 2
            # kv lands first so the PE can start mm1 as early as possible
            for ci in range(CT):
                nc.sync.dma_start(
                    kvb[:, ci : ci + 1, 0:half],
                    kvb_d[p, ci * 128 : (ci + 1) * 128, 0:half],
                )
                nc.scalar.dma_start(
                    qb[:, ci : ci + 1, 0:half],
                    qb_d[p, ci * 128 : (ci + 1) * 128, 0:half],
                )
            for ci in range(CT):
                nc.sync.dma_start(
                    kvb[:, ci : ci + 1, half:CHUNK],
                    kvb_d[p, ci * 128 : (ci + 1) * 128, half:CHUNK],
                )
                nc.scalar.dma_start(
                    qb[:, ci : ci + 1, half:CHUNK],
                    qb_d[p, ci * 128 : (ci + 1) * 128, half:CHUNK],
                )
            kc = po_kc.tile([128, KT, KCS], f8, name="kc")
            nc.sync.dma_start(
                kc[:].rearrange("part a b -> part (a b)"), kc_d[p, :, :]
            )
            # qt in (q, c): SBUF (128 part = q within tile, 16 q-tiles x C)
            qt3 = qt[:].rearrange("part (t c) -> part t c", c=C)
            qtd3 = qt_d[p].rearrange("(t part) c -> part t c", part=128)
            nc.sync.dma_start(qt3[:], qtd3[:])
            chunk_tiles[p] = (qb, kvb, qt, kc)

        emit_loads(0)

        for p in range(PPC):
            qb, kvb, qt, kc = chunk_tiles[p]
            if p + 1 < PPC:
                emit_loads(p + 1)

            outs = po_out.tile([128, QT * C], bf16, name="outs")
            # P^T for the whole chunk: [k-in-tile, k-tile, q] fp8
            pt = po_pt.tile([128, KT, CHUNK], f8, name="pt")

            # ---- mm1 (j-outer: one weight load, 4 panel matmuls) + exp
            for j in range(KT):
                for pp in range(2):  # pan pairs (0,1) and (2,3)
                    psS = po_psS.tile([128, 2, PAN], f32, name="psS")
                    for i in range(2):
                        pan = 2 * pp + i
                        nc.tensor.matmul(
                            psS[:, i : i + 1, :],
                            lhsT=kvb[:, :, j * 128 : (j + 1) * 128],
                            rhs=qb[:, :, pan * PAN : (pan + 1) * PAN],
                            start=True,
                            stop=True,
                            perf_mode=DR,
                        )
                    dst = pt[
                        :, j : j + 1, pp * 1024 : (pp + 1) * 1024
                    ].rearrange("part one (a q) -> part (one a) q", a=2)
                    if _use_dve(j, pp):
                        nc.vector._custom_dve(
                            expi8,
                            out=dst.bitcast(i8),
                            in0=psS[:],
                            s0=EXPI8_S0,
                            s1=EXPI8_S1,
                        )
                    else:
                        nc.scalar.activation(
                            dst, psS[:], Exp, bias=ebias[:], scale=SCALE
                        )

            # ---- mm2 DoubleRow + normalize + fuse, all in (q, c) layout
            for t in range(QT):
                psO = po_psO.tile([128, C + 1], f32, name="psO")
                rot = (2 * (t + 1)) % JP
                jseq = [(rot + i) % JP for i in range(JP)]
                for idx, jp in enumerate(jseq):
                    nc.tensor.matmul(
                        psO[:],
                        lhsT=pt[:, 2 * jp : 2 * jp + 2, t * 128 : (t + 1) * 128],
                        rhs=kc[:, 2 * jp : 2 * jp + 2, 0 : C + 1],
                        start=(idx == 0),
                        stop=(idx == JP - 1),
                        perf_mode=DR,
                    )
                r = po_r.tile([128, 1], f32, name="r")
                nc.vector.reciprocal(r[:], psO[:, C : C + 1])
                g = po_g.tile([128, C], bf16, name="g")
                # kc ones column is 1/w1, so r = w1/rowsum already
                nc.vector.tensor_scalar(g[:], psO[:, 0:C], r[:], None, op0=mult)
                # qt is pre-scaled by w0 on host, so the fuse is a plain add
                nc.gpsimd.tensor_tensor(
                    outs[:, t * C : (t + 1) * C],
                    qt[:, t * C : (t + 1) * C],
                    g[:],
                    op=add,
                )

                # store per panel (4 q-tiles); per-tile on the last panel
                if (t + 1) % 4 == 0:
                    o3 = outs[:].rearrange("part (tt c) -> part tt c", c=C)
                    od3 = out_d[p].rearrange("(tt part) c -> part tt c", part=128)
                    t0 = t - 3
                    if p == PPC - 1 and t == QT - 1:
                        for dt in range(4):
                            nc.sync.dma_start(
                                od3[:, t0 + dt : t0 + dt + 1, :],
                                o3[:, t0 + dt : t0 + dt + 1, :],
                            )
                    else:
                        nc.sync.dma_start(
                            od3[:, t0 : t0 + 4, :], o3[:, t0 : t0 + 4, :]
                        )

    nc.compile()
    return nc


def _get_nc(w0: float, w1: float):
    key = (round(float(w0), 9), round(float(w1), 9))
    if key not in _BUILD_CACHE:
        _BUILD_CACHE[key] = _build(*key)
    return _BUILD_CACHE[key]


def _pairs(arr: np.ndarray) -> np.ndarray:
    # (B, C, H, W) -> (PAIRS, C, CHUNK)
    return (
        arr.reshape(B, C, NCHUNKS, CHUNK)
        .transpose(0, 2, 1, 3)
        .reshape(PAIRS, C, CHUNK)
    )


def _unshard_qc(per_core: list[np.ndarray]) -> np.ndarray:
    # per-core (PPC, CHUNK, C) in (q, c) layout -> (B, C, H, W)
    pairs = np.concatenate(per_core, axis=0)  # (PAIRS, CHUNK, C)
    return np.ascontiguousarray(
        pairs.reshape(B, NCHUNKS, CHUNK, C)
        .transpose(0, 3, 1, 2)
        .reshape(B, C, H, W)
    )


def run(lidar_features, image_features, modality_weights, trace=False):
    import ml_dtypes

    from concourse import bass_utils

    f8 = ml_dtypes.float8_e4m3

    mw = np.asarray(modality_weights, dtype=np.float64)
    e = np.exp(mw - mw.max())
    wsm = e / e.sum()
    w0, w1 = float(wsm[0]), float(wsm[1])

    nc = _get_nc(w0, w1)

    qp = _pairs(np.asarray(image_features, dtype=np.float32))
    kvp = _pairs(np.asarray(lidar_features, dtype=np.float32))
    qpb = qp.astype(f8)
    kvpb = kvp.astype(f8)
    # w0*Q in (q, c) layout (bf16) for the fuse term (pre-scaled on host so
    # the device fuse is a single add)
    qpt = np.ascontiguousarray(qp.transpose(0, 2, 1) * w0).astype(
        ml_dtypes.bfloat16
    )
    # pre-packed (w1*KV) (k, c) tiles + ones column, exactly the kc SBUF
    # layout: psO = P @ (w1*KV) | rowsum, so g = psO * recip(rowsum) needs
    # no extra w1 multiply.
    kcp = np.zeros((PAIRS, 128, KT, KCS), dtype=f8)
    kvw = (kvp * w1).astype(f8)
    # kc[pair, k_in_tile, j, c] = w1 * KV[pair, c, j*128 + k_in_tile]
    kcp[:, :, :, 0:C] = kvw.reshape(PAIRS, C, KT, 128).transpose(0, 3, 2, 1)
    kcp[:, :, :, C] = 1.0
    kcp = kcp.reshape(PAIRS, 128, KT * KCS)
    in_maps = [
        {
            "qt_sh": np.ascontiguousarray(qpt[i * PPC : (i + 1) * PPC]),
            "qb_sh": np.ascontiguousarray(qpb[i * PPC : (i + 1) * PPC]),
            "kvb_sh": np.ascontiguousarray(kvpb[i * PPC : (i + 1) * PPC]),
            "kc_sh": np.ascontiguousarray(kcp[i * PPC : (i + 1) * PPC]),
        }
        for i in range(NCORES)
    ]
    res = bass_utils.run_bass_kernel_spmd(
        nc, in_maps, core_ids=list(range(NCORES)), trace=trace
    )
    out = _unshard_qc(
        [res.results[i]["out_sh"].astype(np.float32) for i in range(NCORES)]
    )
    return out, res


def kernel(lidar_features, image_features, modality_weights) -> np.ndarray:
    out, _ = run(lidar_features, image_features, modality_weights, trace=False)
    return out


# revision 14
# speedup vs baseline: 1.2333x; 1.2333x over previous
"""Trainium2 Bass kernel for CrossAttentionFusion (fp8 DoubleRow version).

Reference computation (shapes hardcoded):
  B=4, C=256, H=W=128, N=16384, CHUNK=2048, nchunks=8.
  q  = image_features  reshaped to (B, nchunks, CHUNK, C)
  kv = lidar_features  reshaped to (B, nchunks, CHUNK, C)
  per (b, chunk): out = softmax(q @ kv.T / sqrt(C)) @ kv
  output = w0 * image_features + w1 * fused,  w = softmax(modality_weights)

Sharding: the 32 independent (b, chunk) pairs are split 4-per-core across
8 NeuronCores (data parallel over batch x chunk; no communication).

fp8 strategy: Q/KV quantized to fp8e4 (e4m3, max 240) on host; both
matmuls run in MatmulPerfMode.DoubleRow (K=256 per instruction, 2 fp8
MACs/cell/cycle).  P = exp(s/sqrt(C) - 3) is produced in fp8: the -3
bias keeps max P ~17 < 240 and cancels in the softmax ratio because the
row-sum is scaled identically.

Per-core structure, per (b, chunk) pair:
  1. mm1 j-outer: for each k-tile j, ONE DoubleRow weight load (KV_j),
     then 4 panel matmuls -> S^T in PSUM (2-bank [128,2,512] tiles).
  2. exp split across two engines: most tiles on ACT (exp -> fp8,
     1024-wide batched), the rest on DVE via a Schraudolph integer exp
     (custom DVE op relu(x*C0+C1) -> int8, bitcast to fp8e4).
  3. mm2 DoubleRow: (q=128p, 257f) = P^T.T @ [KV | 1/w1] over 8 j-pairs.
     The ones column is pre-scaled by 1/w1 so recip gives w1/rowsum.
  4. Epilogue: DVE recip + normalize (g, bf16); GpSimd fuse
     out = qt*w0 + g (qt bf16); DMA out per panel.
"""

import numpy as np

B, C, H, W = 4, 256, 128, 128
N = H * W
CHUNK = 2048
NCHUNKS = N // CHUNK         # 8
NCORES = 8
PAIRS = B * NCHUNKS          # 32
PPC = PAIRS // NCORES        # 4 pairs (chunks) per core
CT = C // 128                # 2 c-tiles
KT = CHUNK // 128            # 16 k-tiles
QT = CHUNK // 128            # 16 q-tiles
PAN = 512                    # q panel width
NPAN = CHUNK // PAN          # 4 panels
KCS = 272                    # kc tile stride (257 cols used)
JP = KT // 2                 # 8 j-pairs for DoubleRow mm2
SCALE = 1.0 / float(np.sqrt(C))
EBIAS = -3.0                 # exp bias: keeps max P < fp8e4 max (240)
LOG2E_8 = 8.0 / float(np.log(2.0))          # 2^3 / ln2 for e4m3 int exp
EXPI8_S0 = SCALE * LOG2E_8                  # psS scale
EXPI8_SHIFT = -0.4                          # Schraudolph centering
EXPI8_S1 = 56.0 + EBIAS * LOG2E_8 + EXPI8_SHIFT

_BUILD_CACHE = {}
_EXPI8_OP = None


def _register_expi8():
    """Register a custom DVE op: out = relu(in*C0 + C1), written as int8.

    With C0/C1 set per the Schraudolph trick, the int8 bit pattern IS
    exp(scale*s + bias) in fp8e4 (bitcast), so the DVE can compute exp
    tiles in a single pass and share the softmax work with ACT.
    """
    global _EXPI8_OP
    if _EXPI8_OP is not None:
        return _EXPI8_OP
    import concourse.dve_ops as dve_ops
    from concourse.bass import dve_ver_for
    from concourse.dve_spec import C0, C1, Spec, Src0, lower, relu
    from concourse.dve_uop import DveOpSpec

    name = "EXP_I8_SCHRAUDOLPH"
    for op in dve_ops.OPS:
        if op.name == name:
            _EXPI8_OP = op
            return op

    spec = Spec(
        body=relu(Src0 * C0 + C1),
        reference=lambda in0, in1, c0, c1, c2: np.maximum(
            in0.astype(np.float32) * c0 + c1, 0.0
        ),
    )
    opcode = dve_ops._CUSTOM_DVE_ROW_BASE + len(dve_ops.OPS)
    shas = {}
    for ver in ("v3", "v4"):
        try:
            uops = lower(spec, ver=ver)
            shas[ver] = DveOpSpec(
                name=name, opcode=opcode, uops=uops, rd1_en=False
            ).sha(ver)
        except Exception:
            pass
    op = dve_ops.DveOp(name, spec, subdim=False, uops_sha=shas)
    dve_ops.OPS.append(op)
    dve_ops._SUB_OPCODE_FOR_NAME[name] = opcode
    _EXPI8_OP = op
    return op


def _use_dve(jj: int, pan: int) -> bool:
    # ~2/7 of the 32 (j-pair, pan) exp tiles per pair go to the DVE
    return (2 * pan + jj) % 7 >= 5


def _build(w0: float, w1: float):
    from contextlib import ExitStack

    import concourse.bass as bass
    import concourse.tile as tile
    from concourse import bacc, mybir

    expi8 = _register_expi8()

    f32 = mybir.dt.float32
    bf16 = mybir.dt.bfloat16
    f8 = mybir.dt.float8e4
    i8 = mybir.dt.int8
    DR = mybir.MatmulPerfMode.DoubleRow
    Exp = mybir.ActivationFunctionType.Exp
    mult = mybir.AluOpType.mult
    add = mybir.AluOpType.add

    nc = bacc.Bacc("TRN2", target_bir_lowering=False, debug=False)
    qt_d = nc.dram_tensor("qt_sh", (PPC, CHUNK, C), bf16, kind="ExternalInput")
    qb_d = nc.dram_tensor("qb_sh", (PPC, C, CHUNK), f8, kind="ExternalInput")
    kvb_d = nc.dram_tensor("kvb_sh", (PPC, C, CHUNK), f8, kind="ExternalInput")
    kc_d = nc.dram_tensor("kc_sh", (PPC, 128, KT * KCS), f8, kind="ExternalInput")
    out_d = nc.dram_tensor("out_sh", (PPC, CHUNK, C), bf16, kind="ExternalOutput")

    with ExitStack() as ctx:
        tc = ctx.enter_context(tile.TileContext(nc))
        po_qt = ctx.enter_context(tc.tile_pool(name="qt", bufs=2))
        po_qb = ctx.enter_context(tc.tile_pool(name="qb", bufs=2))
        po_kvb = ctx.enter_context(tc.tile_pool(name="kvb", bufs=2))
        po_kc = ctx.enter_context(tc.tile_pool(name="kc", bufs=2))
        po_pt = ctx.enter_context(tc.tile_pool(name="pt", bufs=2))
        po_out = ctx.enter_context(tc.tile_pool(name="outs", bufs=2))
        po_g = ctx.enter_context(tc.tile_pool(name="g", bufs=4))
        po_r = ctx.enter_context(tc.tile_pool(name="r", bufs=4))
        po_psS = ctx.enter_context(tc.tile_pool(name="psS", bufs=3, space="PSUM"))
        po_psO = ctx.enter_context(tc.tile_pool(name="psO", bufs=2, space="PSUM"))
        po_const = ctx.enter_context(tc.tile_pool(name="consts", bufs=1))

        ebias = po_const.tile([128, 1], f32, name="ebias")
        nc.gpsimd.memset(ebias[:], EBIAS)

        chunk_tiles = {}

        def emit_loads(p):
            qb = po_qb.tile([128, CT, CHUNK], f8, name="qb")
            kvb = po_kvb.tile([128, CT, CHUNK], f8, name="kvb")
            qt = po_qt.tile([128, QT * C], bf16, name="qt")
            half = CHUNK // 2
            # kv lands first so the PE can start mm1 as early as possible
            for ci in range(CT):
                nc.sync.dma_start(
                    kvb[:, ci : ci + 1, 0:half],
                    kvb_d[p, ci * 128 : (ci + 1) * 128, 0:half],
                )
                nc.scalar.dma_start(
                    qb[:, ci : ci + 1, 0:half],
                    qb_d[p, ci * 128 : (ci + 1) * 128, 0:half],
                )
            for ci in range(CT):
                nc.sync.dma_start(
                    kvb[:, ci : ci + 1, half:CHUNK],
                    kvb_d[p, ci * 128 : (ci + 1) * 128, half:CHUNK],
                )
                nc.scalar.dma_start(
                    qb[:, ci : ci + 1, half:CHUNK],
                    qb_d[p, ci * 128 : (ci + 1) * 128, half:CHUNK],
                )
            kc = po_kc.tile([128, KT, KCS], f8, name="kc")
            nc.sync.dma_start(
                kc[:].rearrange("part a b -> part (a b)"), kc_d[p, :, :]
            )
            # qt in (q, c): SBUF (128 part = q within tile, 16 q-tiles x C)
            qt3 = qt[:].rearrange("part (t c) -> part t c", c=C)
            qtd3 = qt_d[p].rearrange("(t part) c -> part t c", part=128)
            nc.sync.dma_start(qt3[:], qtd3[:])
            chunk_tiles[p] = (qb, kvb, qt, kc)

        emit_loads(0)

        for p in range(PPC):
            qb, kvb, qt, kc = chunk_tiles[p]
            if p + 1 < PPC:
                emit_loads(p + 1)

            outs = po_out.tile([128, QT * C], bf16, name="outs")

            for pan in range(NPAN):
                # mm1 DoubleRow + exp -> P^T panel (k-tile major, fp8).
                # psS pairs (j, j+1) in one 2-bank tile so the ACT exp is a
                # single 1024-wide instruction and the DVE integer exp gets
                # a full tile; pairs also match mm2's DoubleRow j-pairs.
                pt = po_pt.tile([128, KT, PAN], f8, name="pt")
                for jj in range(JP):
                    psS = po_psS.tile([128, 2, PAN], f32, name="psS")
                    for i in range(2):
                        j = 2 * jj + i
                        nc.tensor.matmul(
                            psS[:, i : i + 1, :],
                            lhsT=kvb[:, :, j * 128 : (j + 1) * 128],
                            rhs=qb[:, :, pan * PAN : (pan + 1) * PAN],
                            start=True,
                            stop=True,
                            perf_mode=DR,
                        )
                    dst = pt[:, 2 * jj : 2 * jj + 2, :]
                    if _use_dve(jj, pan):
                        nc.vector._custom_dve(
                            expi8,
                            out=dst.bitcast(i8),
                            in0=psS[:],
                            s0=EXPI8_S0,
                            s1=EXPI8_S1,
                        )
                    else:
                        nc.scalar.activation(
                            dst, psS[:], Exp, bias=ebias[:], scale=SCALE
                        )

                # mm2 DoubleRow + normalize + fuse, all in (q, c) layout
                for tq in range(4):
                    t = pan * 4 + tq
                    psO = po_psO.tile([128, C + 1], f32, name="psO")
                    rot = (2 * (tq + 1)) % JP
                    jseq = [(rot + i) % JP for i in range(JP)]
                    for idx, jp in enumerate(jseq):
                        nc.tensor.matmul(
                            psO[:],
                            lhsT=pt[
                                :, 2 * jp : 2 * jp + 2, tq * 128 : (tq + 1) * 128
                            ],
                            rhs=kc[:, 2 * jp : 2 * jp + 2, 0 : C + 1],
                            start=(idx == 0),
                            stop=(idx == JP - 1),
                            perf_mode=DR,
                        )
                    r = po_r.tile([128, 1], f32, name="r")
                    nc.vector.reciprocal(r[:], psO[:, C : C + 1])
                    g = po_g.tile([128, C], bf16, name="g")
                    nc.vector.tensor_scalar(
                        g[:], psO[:, 0:C], r[:], None, op0=mult
                    )
                    # qt is pre-scaled by w0 on host: the fuse is a plain add
                    nc.gpsimd.tensor_tensor(
                        outs[:, t * C : (t + 1) * C],
                        qt[:, t * C : (t + 1) * C],
                        g[:],
                        op=add,
                    )

                # store this panel's q-tiles (rows are (q, C) in DRAM);
                # on the very last panel store per q-tile to shorten the tail
                o3 = outs[:].rearrange("part (tt c) -> part tt c", c=C)
                od3 = out_d[p].rearrange("(tt part) c -> part tt c", part=128)
                t0 = pan * 4
                if p == PPC - 1 and pan == NPAN - 1:
                    for dt in range(4):
                        nc.sync.dma_start(
                            od3[:, t0 + dt : t0 + dt + 1, :],
                            o3[:, t0 + dt : t0 + dt + 1, :],
                        )
                else:
                    nc.sync.dma_start(
                        od3[:, t0 : t0 + 4, :], o3[:, t0 : t0 + 4, :]
                    )

    nc.compile()
    return nc


def _get_nc(w0: float, w1: float):
    key = (round(float(w0), 9), round(float(w1), 9))
    if key not in _BUILD_CACHE:
        _BUILD_CACHE[key] = _build(*key)
    return _BUILD_CACHE[key]


def _pairs(arr: np.ndarray) -> np.ndarray:
    # (B, C, H, W) -> (PAIRS, C, CHUNK)
    return (
        arr.reshape(B, C, NCHUNKS, CHUNK)
        .transpose(0, 2, 1, 3)
        .reshape(PAIRS, C, CHUNK)
    )


def _unshard_qc(per_core: list[np.ndarray]) -> np.ndarray:
    # per-core (PPC, CHUNK, C) in (q, c) layout -> (B, C, H, W)
    pairs = np.concatenate(per_core, axis=0)  # (PAIRS, CHUNK, C)
    return np.ascontiguousarray(
        pairs.reshape(B, NCHUNKS, CHUNK, C)
        .transpose(0, 3, 1, 2)
        .reshape(B, C, H, W)
    )


def run(lidar_features, image_features, modality_weights, trace=False):
    import ml_dtypes

    from concourse import bass_utils

    f8 = ml_dtypes.float8_e4m3

    mw = np.asarray(modality_weights, dtype=np.float64)
    e = np.exp(mw - mw.max())
    wsm = e / e.sum()
    w0, w1 = float(wsm[0]), float(wsm[1])

    nc = _get_nc(w0, w1)

    qp = _pairs(np.asarray(image_features, dtype=np.float32))
    kvp = _pairs(np.asarray(lidar_features, dtype=np.float32))
    qpb = qp.astype(f8)
    kvpb = kvp.astype(f8)
    # w0*Q in (q, c) layout (bf16) for the fuse term (pre-scaled on host so
    # the device fuse is a single add)
    qpt = np.ascontiguousarray(qp.transpose(0, 2, 1) * w0).astype(
        ml_dtypes.bfloat16
    )
    # pre-packed (w1*KV) (k, c) tiles + ones column, exactly the kc SBUF
    # layout: psO = P @ (w1*KV) | rowsum, so g = psO * recip(rowsum) needs
    # no extra w1 multiply.
    kcp = np.zeros((PAIRS, 128, KT, KCS), dtype=f8)
    kvw = (kvp * w1).astype(f8)
    # kc[pair, k_in_tile, j, c] = w1 * KV[pair, c, j*128 + k_in_tile]
    kcp[:, :, :, 0:C] = kvw.reshape(PAIRS, C, KT, 128).transpose(0, 3, 2, 1)
    kcp[:, :, :, C] = 1.0
    kcp = kcp.reshape(PAIRS, 128, KT * KCS)
    in_maps = [
        {
            "qt_sh": np.ascontiguousarray(qpt[i * PPC : (i + 1) * PPC]),
            "qb_sh": np.ascontiguousarray(qpb[i * PPC : (i + 1) * PPC]),
            "kvb_sh": np.ascontiguousarray(kvpb[i * PPC : (i + 1) * PPC]),
            "kc_sh": np.ascontiguousarray(kcp[i * PPC : (i + 1) * PPC]),
        }
        for i in range(NCORES)
    ]
    res = bass_utils.run_bass_kernel_spmd(
        nc, in_maps, core_ids=list(range(NCORES)), trace=trace
    )
    out = _unshard_qc(
        [res.results[i]["out_sh"].astype(np.float32) for i in range(NCORES)]
    )
    return out, res


def kernel(lidar_features, image_features, modality_weights) -> np.ndarray:
    out, _ = run(lidar_features, image_features, modality_weights, trace=False)
    return out
